# revision 9
# baseline (speedup 1.0000x reference)
"""Trainium2 Bass kernel for CausalSelfAttention (no causal mask in reference).

Problem shapes: x [B=2, T=2048, C=1024], H=16 heads, D=64 head dim.
  q/k/v = x @ W{q,k,v}.T ; att = softmax(q k^T / sqrt(D)) ; y = att v
  out = y @ Wp.T + bp

Sharding over 8 NeuronCores: 4 head-groups (4 heads = 256 dims each) x 2
batches.  Core (g, b) computes a partial output for x[b] restricted to head
group g; the host sums the 4 head-group partials per batch and adds bp.

v3 — precision-engineered fp8:
* Per-element noise does NOT wash out in this attention: Y = sum_s w_s V_s
  is itself a zero-mean weighted average, so signal and noise scale
  identically and every fp8 stage costs ~1.3-2% output error on its own
  (budget: 2% absmax).  Therefore:
* QKV projections: hi/lo-COMPENSATED fp8 DoubleRow (x = x_hi + x_lo,
  16W = w_hi + w_lo; q = x_hi w_hi + x_hi w_lo + x_lo w_hi) — bf16-class
  accuracy at 6/8 the bf16 PE cost.  Weights are pre-scaled by 16 so the
  lo residuals stay in e4m3's normal range (at 1x scale W_lo ~ 6e-4
  underflows).  The x16 per projection makes scores x256 (folded into the
  exp scale) and Y x16 (folded into wp).
* Scores: bf16, 2-head row-packed K=64 (base partitions {0,32,64} only —
  no legal per-head K=32-pair layout; fp8 would be inaccurate anyway).
* PV: P in fp8e4 x V hi/lo-compensated, s-pair DoubleRow, ones columns in
  V_hi emit the softmax denominator on PSUM rows 64:128 (aug of V_lo is 0).
* exp: 16.8M/core, PSUM-bound => only ACT + DVE.  ACT does 14/16 s-tiles
  (native Exp, fp8 out); DVE does 2/16 via one-op Schraudolph:
  round(S*scale*8/ln2 + 55.62) as int8 IS the fp8e4 bit pattern of exp
  (HW converts round-to-nearest).
* Output projection bf16; bf16 partials summed on host (+bp).
"""

import numpy as np
import ml_dtypes

import concourse.bass as bass
import concourse.tile as tile
from concourse import mybir
from concourse.bacc import Bacc
from concourse.bass_utils import run_bass_kernel_spmd

BF16 = mybir.dt.bfloat16
F32 = mybir.dt.float32
F8 = mybir.dt.float8e4
I8 = mybir.dt.int8
NP_BF16 = ml_dtypes.bfloat16
NP_F8 = mybir.dt.np(F8)

P = 128
C = 1024
H = 16
D = 64
N_CORES = 8
N_GROUPS = 4              # head groups (tensor parallel)
N_BATCH = 2               # data parallel over B
HL = H // N_GROUPS        # 4 local heads
DL = HL * D               # 256 local head dims
CHUNK = 512               # t-chunk width (one PSUM bank of fp32)

DR = mybir.MatmulPerfMode.DoubleRow
EXP = mybir.ActivationFunctionType.Exp

WSCALE = 16.0             # host pre-scale on Wq/Wk/Wv (see module docstring)
ESCALE = 0.125 / (WSCALE * WSCALE)
SCH_A = ESCALE * 8.0 / np.log(2.0)
SCH_B = 55.62
DVE_SLOTS = (5, 13)       # of every 16 s-tiles, these exp on DVE


def build_program(T: int = 2048) -> bass.Bass:
    KO = C // P            # 8 k-tiles over the C contraction
    KPAIR = KO // 2        # 4 DoubleRow k-pairs
    TT = T // P            # 16 s/t tiles of 128
    NCH = T // CHUNK       # 4 t-chunks
    KP = DL // P           # 2 k-tiles over local head dims (outproj)

    nc = Bacc()
    xhi_d = nc.declare_dram_parameter("xhi", [C, T], F8, isOutput=False)
    xlo_d = nc.declare_dram_parameter("xlo", [C, T], F8, isOutput=False)
    w_d = {
        n: nc.declare_dram_parameter(n, [C, DL], F8, isOutput=False)
        for n in ("wqh", "wql", "wkh", "wkl", "wvh", "wvl")
    }
    wpT_d = nc.declare_dram_parameter("wpT", [DL, C], BF16, isOutput=False)
    out_d = nc.declare_dram_parameter("out", [T, C], BF16, isOutput=True)

    with tile.TileContext(nc) as tc:
        with (
            tc.tile_pool(name="const", bufs=1) as cp,
            tc.tile_pool(name="att_s", bufs=2, space="PSUM") as att_s,
            tc.tile_pool(name="accy", bufs=2, space="PSUM") as accy,
            tc.tile_pool(name="accps", bufs=2, space="PSUM") as accps,
            tc.tile_pool(name="expp", bufs=32) as exp_pool,
            tc.tile_pool(name="normp", bufs=4) as norm_pool,
            tc.tile_pool(name="outp", bufs=4) as out_pool,
        ):
            xhi_sb = cp.tile([P, KO, T], F8)
            xlo_sb = cp.tile([P, KO, T], F8)
            w_sb = {n: cp.tile([P, KO, DL], F8, name=n) for n in w_d}
            wpT_sb = cp.tile([P, KP, C], BF16)
            # q/k bf16 (x16), m-tile hp rows = heads {2hp, 2hp+1}
            QT_sb = cp.tile([P, KP, T], BF16)
            KT_sb = cp.tile([P, KP, T], BF16)
            # per (s-tile, head): 64 V columns then 64 aug columns;
            # V_hi aug = 1 (denominator), V_lo aug = 0
            Vhi_sb = cp.tile([P, TT, HL, P], F8)
            Vlo_sb = cp.tile([P, TT, HL, P], F8)
            YT_sb = cp.tile([P, KP, T], BF16)

            # dummy matmuls on a memset tile warm the PE clock ramp
            warm_sb = cp.tile([P, CHUNK], BF16)
            nc.vector.memset(warm_sb, 0.0)
            for _w in range(2):
                ps_w = accps.tile([P, CHUNK], F32, tag="acc", name="ps_w")
                nc.tensor.matmul(
                    ps_w, lhsT=warm_sb[:, 0:P], rhs=warm_sb, start=True, stop=True
                )

            # DMAs ordered by first use: K weights, x, Q/V weights, Wp last
            def w_dma(eng, name):
                eng.dma_start(
                    out=w_sb[name][:, :, :],
                    in_=w_d[name][:, :].rearrange("(ko p) d -> p ko d", p=P),
                )

            w_dma(nc.gpsimd, "wkh")
            w_dma(nc.gpsimd, "wkl")
            dma_engs = [nc.sync, nc.gpsimd]
            for x_d, x_sb in ((xhi_d, xhi_sb), (xlo_d, xlo_sb)):
                xT_r = x_d[:, :].rearrange("(ko p) t -> ko p t", p=P)
                for ch in range(NCH):
                    for k in range(KO):
                        dma_engs[ch % 2].dma_start(
                            out=x_sb[:, k, ch * CHUNK : (ch + 1) * CHUNK],
                            in_=xT_r[k][:, ch * CHUNK : (ch + 1) * CHUNK],
                        )
            for n in ("wqh", "wql", "wvh", "wvl"):
                w_dma(nc.sync, n)
            nc.sync.dma_start(
                out=wpT_sb[:, :, :],
                in_=wpT_d[:, :].rearrange("(kp p) n -> p kp n", p=P),
            )

            nc.gpsimd.memset(Vhi_sb[:, :, :, D : 2 * D], 1.0)
            nc.gpsimd.memset(Vlo_sb[:, :, :, D : 2 * D], 0.0)

            # ---------- emission helpers ----------
            CHAINS = {  # compensated product: hi*hi + hi*lo + lo*hi
                "q": [(xhi_sb, "wqh"), (xhi_sb, "wql"), (xlo_sb, "wqh")],
                "k": [(xhi_sb, "wkh"), (xhi_sb, "wkl"), (xlo_sb, "wkh")],
                "v": [(xhi_sb, "wvh"), (xhi_sb, "wvl"), (xlo_sb, "wvh")],
            }

            def emit_qk_group(which, o_sb, j, ch, on_act):
                ps = accps.tile([P, CHUNK], F32, tag="acc", name="ps")
                n_mm = 3 * KPAIR
                i = 0
                for xs, wn in CHAINS[which]:
                    for kk in range(KPAIR):
                        nc.tensor.matmul(
                            ps,
                            lhsT=w_sb[wn][:, 2 * kk : 2 * kk + 2, j * P : (j + 1) * P],
                            rhs=xs[:, 2 * kk : 2 * kk + 2, ch * CHUNK : (ch + 1) * CHUNK],
                            start=(i == 0),
                            stop=(i == n_mm - 1),
                            perf_mode=DR,
                        )
                        i += 1
                dst = o_sb[:, j, ch * CHUNK : (ch + 1) * CHUNK]
                if on_act:
                    nc.scalar.copy(out=dst, in_=ps)
                else:
                    nc.vector.tensor_copy(out=dst, in_=ps)

            def emit_v_group(m):
                ps = accps.tile([P, CHUNK], F32, tag="acc", name="ps")
                n_mm = 3 * KPAIR
                i = 0
                for xs, wn in CHAINS["v"]:
                    for kk in range(KPAIR):
                        nc.tensor.matmul(
                            ps[:, 0:DL],
                            lhsT=xs[:, 2 * kk : 2 * kk + 2, m * P : (m + 1) * P],
                            rhs=w_sb[wn][:, 2 * kk : 2 * kk + 2, :],
                            start=(i == 0),
                            stop=(i == n_mm - 1),
                            perf_mode=DR,
                        )
                        i += 1
                vin = ps[:, 0:DL].rearrange("p (h e) -> p h e", e=D)
                nc.vector.tensor_copy(out=Vhi_sb[:, m, :, 0:D], in_=vin)
                nc.vector.tensor_tensor(
                    out=Vlo_sb[:, m, :, 0:D],
                    in0=vin,
                    in1=Vhi_sb[:, m, :, 0:D],
                    op=mybir.AluOpType.subtract,
                )

            exps = {}  # (ch, hp) -> list of 8 E tiles [P, 2, 2*CHUNK] fp8

            def emit_sexp(ch, hp):
                t0 = ch * CHUNK
                lst = []
                for s in range(TT):
                    ps_s = att_s.tile([P, 2 * CHUNK], F32, tag="s", name="ps_s")
                    for ha in range(2):
                        nc.tensor.matmul(
                            ps_s[:, ha * CHUNK : (ha + 1) * CHUNK],
                            lhsT=KT_sb[ha * D : (ha + 1) * D, hp, s * P : (s + 1) * P],
                            rhs=QT_sb[ha * D : (ha + 1) * D, hp, t0 : t0 + CHUNK],
                            start=True,
                            stop=True,
                        )
                    if s % 2 == 0:
                        E = exp_pool.tile([P, 2, 2 * CHUNK], F8, tag="e", name="E")
                        lst.append(E)
                    dst = lst[-1][:, s % 2, :]
                    if (s % 16) in DVE_SLOTS:
                        nc.vector.tensor_scalar(
                            out=dst.bitcast(I8),
                            in0=ps_s,
                            scalar1=SCH_A,
                            scalar2=SCH_B,
                            op0=mybir.AluOpType.mult,
                            op1=mybir.AluOpType.add,
                        )
                    else:
                        nc.scalar.activation(out=dst, in_=ps_s, func=EXP, scale=ESCALE)
                exps[(ch, hp)] = lst

            def emit_pv(ch, hp):
                t0 = ch * CHUNK
                lst = exps.pop((ch, hp))
                JJ = TT // 2
                for ha in range(2):
                    h = hp * 2 + ha
                    ps_y = accy.tile([P, CHUNK], F32, tag="y", name="ps_y")
                    i = 0
                    for v_sb in (Vhi_sb, Vlo_sb):
                        for jj in range(JJ):
                            nc.tensor.matmul(
                                ps_y,
                                lhsT=v_sb[:, 2 * jj : 2 * jj + 2, h, :],
                                rhs=lst[jj][:, :, ha * CHUNK : (ha + 1) * CHUNK],
                                start=(i == 0),
                                stop=(i == 2 * JJ - 1),
                                perf_mode=DR,
                            )
                            i += 1
                    recip = norm_pool.tile([D, CHUNK], F32, tag="r", name="recip")
                    nc.vector.reciprocal(out=recip, in_=ps_y[D : 2 * D, :])
                    nc.vector.tensor_mul(
                        out=YT_sb[ha * D : (ha + 1) * D, hp, t0 : t0 + CHUNK],
                        in0=ps_y[0:D, :],
                        in1=recip,
                    )

            def emit_outproj(ch, last=False):
                for mt in range(CHUNK // P):
                    m = ch * (CHUNK // P) + mt
                    for n2 in range(C // CHUNK):
                        ps_o = accps.tile([P, CHUNK], F32, tag="acc", name="ps_o")
                        for kk in range(KP):
                            nc.tensor.matmul(
                                ps_o,
                                lhsT=YT_sb[:, kk, m * P : (m + 1) * P],
                                rhs=wpT_sb[:, kk, n2 * CHUNK : (n2 + 1) * CHUNK],
                                start=(kk == 0),
                                stop=(kk == KP - 1),
                            )
                        o_sb = out_pool.tile([P, CHUNK], BF16, tag="o", name="o_sb")
                        # in the tail the exp stream is done, so the ACT
                        # engine is free to take the drain copies
                        if last:
                            nc.scalar.copy(out=o_sb, in_=ps_o)
                        else:
                            nc.vector.tensor_copy(out=o_sb, in_=ps_o)
                        dma_engs[n2 % 2].dma_start(
                            out=out_d[
                                m * P : (m + 1) * P,
                                n2 * CHUNK : (n2 + 1) * CHUNK,
                            ],
                            in_=o_sb,
                        )

            # ---------- emission order ----------
            # K projection first (scores need all of KT), copies on the
            # still-idle ACT engine; then Q(ch0) and the first two score+exp
            # streams; V and remaining Q interleave as PE filler; outproj
            # lags one chunk as PE filler for the exp-paced stretches.
            for ch in range(NCH):
                emit_qk_group("k", KT_sb, 0, ch, on_act=True)
            for ch in range(NCH):
                emit_qk_group("k", KT_sb, 1, ch, on_act=True)
            emit_qk_group("q", QT_sb, 0, 0, on_act=True)
            emit_qk_group("q", QT_sb, 1, 0, on_act=True)
            emit_sexp(0, 0)
            emit_sexp(0, 1)
            vq = []
            for m in range(TT // 2):
                vq.append(("v", m))
            vq.append(("q", (0, 1)))
            vq.append(("sexp", (1, 0)))
            for m in range(TT // 2, TT):
                vq.append(("v", m))
            vq.append(("q", (1, 1)))
            vq.append(("sexp", (1, 1)))
            qrest = [(j, ch) for ch in range(2, NCH) for j in range(2)]
            mixed = []
            vi = 0
            for item in vq:
                mixed.append(item)
                if item[0] == "v":
                    vi += 1
                    if vi % 3 == 0 and qrest:
                        mixed.append(("q", qrest.pop(0)))
            for kind, arg in mixed:
                if kind == "v":
                    emit_v_group(arg)
                elif kind == "q":
                    emit_qk_group("q", QT_sb, arg[0], arg[1], on_act=False)
                else:
                    emit_sexp(arg[0], arg[1])
            for j, ch in qrest:
                emit_qk_group("q", QT_sb, j, ch, on_act=False)
            for ch in range(NCH):
                if 2 <= ch + 1 < NCH:
                    emit_sexp(ch + 1, 0)
                emit_pv(ch, 0)
                if ch >= 1:
                    emit_outproj(ch - 1)
                emit_pv(ch, 1)
                if 2 <= ch + 1 < NCH:
                    emit_sexp(ch + 1, 1)
            emit_outproj(NCH - 1, last=True)
    nc.finalize()
    return nc


def shard_inputs(x, Wk, Wq, Wv, Wp, T=2048):
    """Build the 8 per-core input dicts (hi/lo fp8 splits, host scaling)."""
    x = np.asarray(x, np.float32)
    Wk = np.asarray(Wk, np.float32)
    Wq = np.asarray(Wq, np.float32)
    Wv = np.asarray(Wv, np.float32)
    Wp = np.asarray(Wp, np.float32)

    def split8(a):
        hi = a.astype(NP_F8)
        lo = (a - hi.astype(np.float32)).astype(NP_F8)
        return hi, lo

    xs = []
    for b in range(x.shape[0]):
        xT = np.ascontiguousarray(x[b, :T].T)
        hi, lo = split8(xT)
        xs.append((np.ascontiguousarray(hi), np.ascontiguousarray(lo)))

    in_maps = []
    for g in range(N_GROUPS):
        sl = slice(g * DL, (g + 1) * DL)
        m = {}
        for n, W in (("wq", Wq), ("wk", Wk), ("wv", Wv)):
            hi, lo = split8(np.ascontiguousarray(W[sl].T * WSCALE))
            m[n + "h"] = np.ascontiguousarray(hi)
            m[n + "l"] = np.ascontiguousarray(lo)
        m["wpT"] = np.ascontiguousarray((Wp[:, sl].T / WSCALE).astype(NP_BF16))
        for b in range(len(xs)):
            im = dict(m)
            im["xhi"], im["xlo"] = xs[b]
            in_maps.append(im)
    return in_maps


_PROGRAM = None


def kernel(x, Wk, Wq, Wv, Wp, bp):
    global _PROGRAM
    x = np.asarray(x, np.float32)
    bp = np.asarray(bp, np.float32)
    B, T, _ = x.shape

    if _PROGRAM is None:
        _PROGRAM = build_program(T)
    nc = _PROGRAM

    in_maps = shard_inputs(x, Wk, Wq, Wv, Wp, T=T)
    res = run_bass_kernel_spmd(nc, in_maps, core_ids=list(range(N_CORES)))
    parts = [r["out"] for r in res.results]

    out = np.zeros((B, T, C), np.float32)
    for g in range(N_GROUPS):
        for b in range(B):
            out[b] += parts[g * N_BATCH + b].astype(np.float32)
    out += bp
    return out
